# revision 71
# baseline (speedup 1.0000x reference)
"""Trainium2 Bass kernel for the sparse-attention nn module (nn_BDH_48421461295735).

Strategy: 8 NeuronCores = 8 (batch, head) pairs (B=2 x NH=4).  Each core runs
all 4 layers for its head; the only cross-core traffic is a per-layer
AllReduce (within each batch's group of 4 cores) of the per-head decoder
partial p = (x_sparse*y_sparse) @ dec_h, chunked by 512 token rows (bf16).

v2 changes over the first working version:
  - score matmul in fp8e4 with DoubleRow perf mode: qrT stored as pair
    tiles qp[j] = [128, 2, T] (even half / odd half of each rope pair
    group), contracting 256 rows per matmul -> ~1.6x fewer PE cycles on
    the dominant score computation.
  - x_sparse is stashed (bf16, per-chunk double buffer) by the p1 relu,
    so p3 no longer recomputes it (saves 2 matmuls + relu per k-tile).
  - all 128x128 transposes (xT, ykvT) go through the DMA XBAR
    (dma_start transpose=True) instead of PE transpose + DVE copy; the
    host also supplies x0 pre-transposed so setup is pure DMA.
  - LN is fused: stats via DVE reduce + ACT Square(bias=-mu, accum) and
    the apply via ACT Identity(scale=1/sd, bias=-mu/sd); relu is fused
    into the xy product via scalar_tensor_tensor(max 0, mult).
  - AllReduce in bf16; p4 (x update) is emitted one full chunk behind
    through a flattened (layer, chunk) pipeline so the collective
    latency never heads-of-line-blocks the vector queues; final lm_head
    matmuls are interleaved per chunk.
"""

import math
import sys

import numpy as np

for _p in ("/opt/trn_rl_repo",):
    if _p not in sys.path:
        sys.path.insert(0, _p)

import concourse.bass as bass
import concourse.bacc as bacc
import concourse.mybir as mybir
import concourse.tile as tile
from concourse.bass_utils import run_bass_kernel_spmd

F32 = mybir.dt.float32
BF16 = mybir.dt.bfloat16
FP8 = mybir.dt.float8e4
AF = mybir.ActivationFunctionType
ALU = mybir.AluOpType
DR = mybir.MatmulPerfMode.DoubleRow

FULL_CFG = dict(T=2048, D=256, N=2048, NL=4, V=256, NH=4, B=2)
P = 128
SUP = 512
EPS = 1e-5
YKV_SC = 64.0  # ykvT pre-scale so fp8e4 never saturates (folded into rs4)

SCORE_FP8 = True  # score + y_sparse matmuls via fp8e4 DoubleRow


def build_nc(cfg, mm_dt=BF16, n_cores=8, score_fp8=SCORE_FP8):
    T, D, N, NL, V = cfg["T"], cfg["D"], cfg["N"], cfg["NL"], cfg["V"]
    NH = cfg["NH"]
    assert T % SUP == 0 and D % P == 0 and N % 256 == 0 and V == D
    nTB, nTS, nD, nK = T // P, T // SUP, D // P, N // P
    nJ = nK // 2
    nQ = SUP // P  # 4
    sc_dt = FP8 if score_fp8 else mm_dt

    nc = bacc.Bacc("TRN2", target_bir_lowering=False, debug=False,
                   num_devices=n_cores)

    x0_d = nc.dram_tensor("x0", [T, D], F32, kind="ExternalInput")
    x0b_d = nc.dram_tensor("x0b", [T, D], mm_dt, kind="ExternalInput")
    x0t_d = nc.dram_tensor("x0t", [D, T], mm_dt, kind="ExternalInput")
    wenc_d = nc.dram_tensor("wenc", [D, N], mm_dt, kind="ExternalInput")
    wencv_d = nc.dram_tensor("wencv", [P, 2 * N], mm_dt, kind="ExternalInput")
    wdec_d = nc.dram_tensor("wdec", [N, D], mm_dt, kind="ExternalInput")
    wlm_d = nc.dram_tensor("wlm", [D, V], mm_dt, kind="ExternalInput")
    ctab_d = nc.dram_tensor("ctab", [N // 2, T], mm_dt, kind="ExternalInput")
    stab_d = nc.dram_tensor("stab", [N // 2, T], mm_dt, kind="ExternalInput")
    maskt_d = nc.dram_tensor("maskt", [P, P], mm_dt, kind="ExternalInput")
    out_d = nc.dram_tensor("out", [T, V], F32, kind="ExternalOutput")

    # AllReduce groups: one group of NH cores per batch.
    RG = [list(range(g * NH, (g + 1) * NH)) for g in range(max(1, n_cores // NH))]

    with tile.TileContext(nc) as tc:
        _keep = []  # keep tc.tile free-closures alive (GC would release pools)

        def ptile(shape, dtype, name, **kw):
            t, free = tc.tile(shape, dtype, name=name, **kw)
            _keep.append(free)
            return t

        # ---- persistent SBUF tensors ----
        wenc_sb = [ptile([P, N], mm_dt, name=f"wenc{d}") for d in range(nD)]
        wencvp = ptile([P, 2, N], mm_dt, name="wencvp")  # d-pair layout
        wdec_sb = [ptile([P, D], mm_dt, name=f"wdec{k}") for k in range(nK)]
        wlm_sb = [ptile([P, V], mm_dt, name=f"wlm{d}") for d in range(nD)]
        maskt_sb = ptile([P, P], mm_dt, name="maskt")
        x_f32 = [ptile([P, D], F32, name=f"xf{t}") for t in range(nTB)]
        # x_bf / xT double-buffered by layer parity (p4 writes the other one)
        x_bf = [[ptile([P, D], mm_dt, name=f"xb{pp}_{t}") for t in range(nTB)]
                for pp in range(2)]
        xT_bf = [[ptile([P, T], mm_dt, name=f"xT{pp}_{d}") for d in range(nD)]
                 for pp in range(2)]
        # qrT pair tiles: [:,0,:] = even half (pair group j), [:,1,:] = odd
        qp = [ptile([P, 2, T], sc_dt, name=f"qp{j}") for j in range(nJ)]
        # ykvT in fp8 d-pairs (scaled by 1/YKV_SC; the scale commutes out
        # through relu/decoder and is folded into rs4)
        ykvTp = ptile([P, 2, T], mm_dt, name="ykvTp")
        # x_sparse stash, double-buffered by chunk parity
        xs_sb = [[ptile([P, SUP], mm_dt, name=f"xs{g}_{k}") for k in range(nK)]
                 for g in range(2)]
        eps_sb = ptile([P, 1], F32, name="epsb")
        nc.vector.memset(eps_sb[:], EPS)
        epsk_sb = ptile([P, 1], F32, name="epskb")
        nc.vector.memset(epsk_sb[:], EPS / (YKV_SC * YKV_SC))
        ones_sb = ptile([P, 1], mm_dt, name="onesb")
        nc.vector.memset(ones_sb[:], 1.0)

        # per-layer DRAM bounce buffers for the chunked AllReduce (bf16)
        p_loc = [ptile([T, D], mm_dt, space="DRAM", name=f"ploc{l}")
                 for l in range(NL)]
        p_sum = [ptile([T, D], mm_dt, space="DRAM", addr_space="Shared",
                       name=f"psum{l}") for l in range(NL)]

        # transient pools
        _cms = [tc.tile_pool(name="spT", bufs=4),     # rope tables
                tc.tile_pool(name="spR", bufs=3),     # rope temporaries
                tc.tile_pool(name="spS", bufs=4),     # st_sb, xy
                tc.tile_pool(name="spC", bufs=2),     # pch/pin/och staging
                tc.tile_pool(name="spL", bufs=8),     # LN scalars + scratch
                tc.tile_pool(name="ppb", bufs=4, space="PSUM"),
                tc.tile_pool(name="ppw", bufs=2, space="PSUM"),
                tc.tile_pool(name="pss", bufs=1, space="PSUM"),
                tc.tile_pool(name="ppr", bufs=1, space="PSUM")]
        spT, spR, spS, spC, spL, ppb, ppw, pss, ppr = \
            [cm.__enter__() for cm in _cms]

        def ln_stats_nm(src_ap):
            """LN stats over free dim D of [P, D] f32/psum AP.
            Returns (nm, rs): per-partition -mu and 1/sd."""
            s1 = spL.tile([P, 1], F32, tag="ln1", name="s1")
            nc.vector.reduce_sum(s1[:], src_ap, axis=mybir.AxisListType.X)
            nm = spL.tile([P, 1], F32, tag="ln2", name="nm")
            nc.vector.tensor_scalar_mul(nm[:], s1[:], -1.0 / D)
            sq = spL.tile([P, D], BF16, tag="lnsq", name="sq", bufs=2)
            ss = spL.tile([P, 1], F32, tag="ln3", name="ss")
            nc.scalar.activation(sq[:], src_ap, AF.Square, bias=nm[:],
                                 accum_out=ss[:])
            sd = spL.tile([P, 1], F32, tag="ln4", name="sd")
            nc.scalar.activation(sd[:], ss[:], AF.Sqrt, bias=eps_sb[:],
                                 scale=1.0 / D)
            rs = spL.tile([P, 1], F32, tag="ln5", name="rs")
            nc.vector.reciprocal(rs[:], sd[:])
            return nm, rs



        # ---- setup: pure DMAs; only what p1(0,0)/p2(0,0) needs goes on the
        # sync queue (which also carries the first rope tables) ----
        with nc.named_scope("setup"):
            for d in range(nD):
                nc.sync.dma_start(wenc_sb[d][:], wenc_d[d * P:(d + 1) * P, :])
            for d in range(nD):
                nc.sync.dma_start(xT_bf[0][d][:], x0t_d[d * P:(d + 1) * P, :])
            for t in range(nTB):
                nc.scalar.dma_start(x_bf[0][t][:], x0b_d[t * P:(t + 1) * P, :])
            nc.scalar.dma_start(
                wencvp[:].rearrange("p a n -> p (a n)"), wencv_d[:, :])
            nc.scalar.dma_start(maskt_sb[:], maskt_d[:, :])
            for k in range(nK):
                nc.scalar.dma_start(wdec_sb[k][:], wdec_d[k * P:(k + 1) * P, :])
            for t in range(nTB):
                nc.scalar.dma_start(x_f32[t][:], x0_d[t * P:(t + 1) * P, :])
            for d in range(nD):
                nc.scalar.dma_start(wlm_sb[d][:], wlm_d[d * P:(d + 1) * P, :])

        def emit_p1(l, ts):
            par = l % 2
            xT = xT_bf[par]
            xs = xs_sb[ts % 2]
            c0, c1 = ts * SUP, (ts + 1) * SUP
            with nc.named_scope(f"l{l}c{ts}_p1"):
                for j in range(nJ):
                    ct = spT.tile([P, SUP], mm_dt, tag="ctc", name="ct")
                    st = spT.tile([P, SUP], mm_dt, tag="stc", name="st")
                    nc.sync.dma_start(ct[:], ctab_d[j * P:(j + 1) * P, c0:c1])
                    nc.sync.dma_start(st[:], stab_d[j * P:(j + 1) * P, c0:c1])
                    psA = ppb.tile([P, SUP], F32, tag="big", name="psA")
                    psB = ppb.tile([P, SUP], F32, tag="big", name="psB")
                    for d in range(nD):
                        nc.tensor.matmul(
                            psA[:], wenc_sb[d][:, j * P:(j + 1) * P],
                            xT[d][:, c0:c1],
                            start=(d == 0), stop=(d == nD - 1))
                    for d in range(nD):
                        nc.tensor.matmul(
                            psB[:],
                            wenc_sb[d][:, (j + nJ) * P:(j + nJ + 1) * P],
                            xT[d][:, c0:c1],
                            start=(d == 0), stop=(d == nD - 1))
                    xsE, xsO = xs[j], xs[j + nJ]
                    nc.scalar.activation(xsE[:], psA[:], AF.Relu)
                    nc.scalar.activation(xsO[:], psB[:], AF.Relu)
                    # rope: qE = relu(A)*c - relu(B)*s ; qO = relu(B)*c + relu(A)*s
                    # DVE ops read at most one SBUF tensor (PSUM+SBUF is ~2x
                    # faster than SBUF+SBUF); gpsimd does the SBUF-only mults.
                    t0 = ppr.tile([P, SUP], F32, tag="tr", name="t0")
                    t1 = spR.tile([P, SUP], mm_dt, tag="t1", name="t1")
                    t2 = spR.tile([P, SUP], mm_dt, tag="t2", name="t2")
                    qE, qO = qp[j][:, 0, c0:c1], qp[j][:, 1, c0:c1]
                    nc.vector.scalar_tensor_tensor(t0[:], psA[:], 0.0, ct[:],
                                                   ALU.max, ALU.mult)
                    nc.gpsimd.tensor_tensor(t1[:], xsO[:], st[:], ALU.mult)
                    nc.vector.tensor_tensor(qE, t0[:], t1[:], ALU.subtract)
                    nc.gpsimd.tensor_tensor(t2[:], xsO[:], ct[:], ALU.mult)
                    t3 = ppr.tile([P, SUP], F32, tag="tr", name="t3")
                    nc.vector.scalar_tensor_tensor(t3[:], psA[:], 0.0, st[:],
                                                   ALU.max, ALU.mult)
                    nc.vector.tensor_tensor(qO, t3[:], t2[:], ALU.add)

        _rs4 = [None]

        def emit_p2(l, ts):
            par = l % 2
            xb = x_bf[par]
            c0, c1 = ts * SUP, (ts + 1) * SUP
            with nc.named_scope(f"l{l}c{ts}_p2"):
                # ykv is computed directly in transposed layout:
                #   ykvT[d, t] = sum_s x[s, d] * scoresT[s, t]
                # The per-token LN of ykv needs no mean (ykv is exactly
                # zero-mean since x is layer-normed), and the 1/sd scale
                # commutes through relu/encv/decoder, so it is applied to
                # p's rows (per-partition scale) in p3's pch copy instead.
                yT_ps = [ppw.tile([P, SUP], F32, tag="wide", name=f"yT{d}")
                         for d in range(nD)]
                nsb = nQ * ts + nQ
                sb_order = list(range(nsb))
                prevs = None
                for si, sb in enumerate(sb_order):
                    r = sb - nQ * ts
                    q0 = max(0, r)
                    st_ps = ppb.tile([P, SUP], F32, tag="big", name="st_ps")
                    dst = st_ps[:, q0 * P:SUP]
                    for j in range(nJ):
                        lhs_t = qp[j][:, :, sb * P:(sb + 1) * P]
                        rhs_t = qp[j][:, :, c0 + q0 * P:c1]
                        if score_fp8:
                            nc.tensor.matmul(
                                dst, lhs_t, rhs_t,
                                start=(j == 0), stop=(j == nJ - 1),
                                perf_mode=DR)
                        else:
                            for h in range(2):
                                nc.tensor.matmul(
                                    dst, lhs_t[:, h, :], rhs_t[:, h, :],
                                    start=(j == 0 and h == 0),
                                    stop=(j == nJ - 1 and h == 1))
                    st_sb = spS.tile([P, SUP], mm_dt, tag="stsb",
                                     name="st_sb")
                    if r >= 0:
                        if r > 0:
                            nc.gpsimd.memset(st_sb[:, 0:r * P], 0.0)
                        nc.vector.tensor_tensor(
                            st_sb[:, r * P:(r + 1) * P],
                            st_ps[:, r * P:(r + 1) * P], maskt_sb[:],
                            ALU.mult)
                        if r + 1 < nQ:
                            nc.scalar.activation(st_sb[:, (r + 1) * P:SUP],
                                                 st_ps[:, (r + 1) * P:SUP],
                                                 AF.Copy)
                    else:
                        nc.scalar.activation(st_sb[:], st_ps[:], AF.Copy)
                    # ykvT matmuls lag one score block so the PE never
                    # waits on the st_sb copy it just requested
                    if prevs is not None:
                        psi, pst, psb = prevs
                        for d in range(nD):
                            nc.tensor.matmul(
                                yT_ps[d][:], xb[psb][:, d * P:(d + 1) * P],
                                pst[:], start=(psi == 0), stop=False)
                    prevs = (si, st_sb, sb)
                psi, pst, psb = prevs
                for d in range(nD):
                    nc.tensor.matmul(
                        yT_ps[d][:], xb[psb][:, d * P:(d + 1) * P],
                        pst[:], start=(psi == 0), stop=True)
                # stats: ss4[:, q] = sum_d ykvT^2 via ones-matmul, then
                # rs4 = 1/sqrt(ss/D + eps) stays in token-column layout.
                sqs = []
                for d in range(nD):
                    sq = spS.tile([P, SUP], mm_dt, tag=f"sq{d}", name="sq",
                                  bufs=2)
                    nc.scalar.activation(sq[:], yT_ps[d][:], AF.Square)
                    nc.scalar.activation(ykvTp[:, d, c0:c1], yT_ps[d][:],
                                         AF.Copy, scale=1.0 / YKV_SC)
                    sqs.append(sq)
                ss4 = pss.tile([P, nQ], F32, tag="ss", name="ss4")
                for q in range(nQ):
                    for d in range(nD):
                        # single accumulation group for the whole bank: one
                        # start pends the 2KB zero region; each column's
                        # first touch write-initializes via has_written.
                        nc.tensor.matmul(
                            ss4[:, q:q + 1], sqs[d][:, q * P:(q + 1) * P],
                            ones_sb[:],
                            start=(q == 0 and d == 0),
                            stop=(q == nQ - 1 and d == nD - 1))
                # sd scaled by 1/YKV_SC so reciprocal yields YKV_SC/sd,
                # undoing the ykvTp pre-scale for free.
                sd4 = spL.tile([P, nQ], F32, tag="sd4", name="sd4")
                nc.scalar.activation(sd4[:], ss4[:], AF.Sqrt, bias=epsk_sb[:],
                                     scale=1.0 / (D * YKV_SC * YKV_SC))
                rs4 = spL.tile([P, nQ], F32, tag="rs4", name="rs4")
                nc.vector.reciprocal(rs4[:], sd4[:])
                _rs4[0] = rs4

        _last_pch = [None]

        def emit_p3(l, ts):
            xs = xs_sb[ts % 2]
            c0, c1 = ts * SUP, (ts + 1) * SUP
            rs4 = _rs4[0]
            with nc.named_scope(f"l{l}c{ts}_p3"):
                p_ps = [ppw.tile([P, SUP], F32, tag="wide", name=f"pp{h}")
                        for h in range(nQ // 2)]
                for k in range(nK):
                    ys_ps = ppb.tile([P, SUP], F32, tag="big", name="ys_ps")
                    for d in range(nD):
                        nc.tensor.matmul(
                            ys_ps[:], wencvp[:, d, k * P:(k + 1) * P],
                            ykvTp[:, d, c0:c1],
                            start=(d == 0), stop=(d == nD - 1))
                    xy = spS.tile([P, SUP], mm_dt, tag="xy", name="xy")
                    # fused relu+mult on DVE (gpsimd cannot read PSUM)
                    nc.vector.scalar_tensor_tensor(
                        xy[:], ys_ps[:], 0.0, xs[k][:], ALU.max, ALU.mult)
                    for q in range(nQ):
                        nc.tensor.matmul(
                            p_ps[q // 2][:, (q % 2) * D:(q % 2 + 1) * D],
                            xy[:, q * P:(q + 1) * P],
                            wdec_sb[k][:],
                            start=(k == 0 and q % 2 == 0),
                            stop=(k == nK - 1 and q % 2 == 1))
                pch = spC.tile([P, nQ * D], mm_dt, tag="pch", name="pch")
                for q in range(nQ):
                    nc.scalar.activation(
                        pch[:, q * D:(q + 1) * D],
                        p_ps[q // 2][:, (q % 2) * D:(q % 2 + 1) * D],
                        AF.Copy, scale=rs4[:, q:q + 1])
                _last_pch[0] = pch

        def emit_p13(l3, ts3, p1_next):
            """p3 of chunk (l3,ts3) interleaved with p1 of the next chunk:
            the PE chews p3's ys/dec matmuls between p1's rope-paced
            groups instead of stalling on the psA ring."""
            xs3 = xs_sb[ts3 % 2]
            c30, c31 = ts3 * SUP, (ts3 + 1) * SUP
            rs4 = _rs4[0]
            if p1_next is not None:
                l1, ts1 = p1_next
                par1 = l1 % 2
                xT1 = xT_bf[par1]
                xs1 = xs_sb[ts1 % 2]
                c10, c11 = ts1 * SUP, (ts1 + 1) * SUP
            with nc.named_scope(f"l{l3}c{ts3}_p31"):
                p_ps = [ppw.tile([P, SUP], F32, tag="wide", name=f"pp{h}")
                        for h in range(nQ // 2)]

                prevk = [None]  # decoder matmuls lag one k behind the STT

                def dec_mms(pk, pxy, stop):
                    for q in range(nQ):
                        nc.tensor.matmul(
                            p_ps[q // 2][:, (q % 2) * D:(q % 2 + 1) * D],
                            pxy[:, q * P:(q + 1) * P],
                            wdec_sb[pk][:],
                            start=(pk == 0 and q % 2 == 0),
                            stop=(stop and q % 2 == 1))

                def p3_step(k):
                    ys_ps = ppb.tile([P, SUP], F32, tag="big", name="ys_ps")
                    for d in range(nD):
                        nc.tensor.matmul(
                            ys_ps[:], wencvp[:, d, k * P:(k + 1) * P],
                            ykvTp[:, d, c30:c31],
                            start=(d == 0), stop=(d == nD - 1))
                    xy = spS.tile([P, SUP], mm_dt, tag="xy", name="xy")
                    nc.vector.scalar_tensor_tensor(
                        xy[:], ys_ps[:], 0.0, xs3[k][:], ALU.max, ALU.mult)
                    if prevk[0] is not None:
                        dec_mms(prevk[0][0], prevk[0][1], stop=False)
                    prevk[0] = (k, xy)

                def p1_step(j):
                    ct = spT.tile([P, SUP], mm_dt, tag="ctc", name="ct")
                    st = spT.tile([P, SUP], mm_dt, tag="stc", name="st")
                    nc.sync.dma_start(ct[:],
                                      ctab_d[j * P:(j + 1) * P, c10:c11])
                    nc.sync.dma_start(st[:],
                                      stab_d[j * P:(j + 1) * P, c10:c11])
                    psA = ppb.tile([P, SUP], F32, tag="big", name="psA")
                    psB = ppb.tile([P, SUP], F32, tag="big", name="psB")
                    for d in range(nD):
                        nc.tensor.matmul(
                            psA[:], wenc_sb[d][:, j * P:(j + 1) * P],
                            xT1[d][:, c10:c11],
                            start=(d == 0), stop=(d == nD - 1))
                    for d in range(nD):
                        nc.tensor.matmul(
                            psB[:],
                            wenc_sb[d][:, (j + nJ) * P:(j + nJ + 1) * P],
                            xT1[d][:, c10:c11],
                            start=(d == 0), stop=(d == nD - 1))
                    xsE, xsO = xs1[j], xs1[j + nJ]
                    nc.scalar.activation(xsE[:], psA[:], AF.Relu)
                    nc.scalar.activation(xsO[:], psB[:], AF.Relu)
                    t0 = ppr.tile([P, SUP], F32, tag="tr", name="t0")
                    t1 = spR.tile([P, SUP], mm_dt, tag="t1", name="t1")
                    t2 = spR.tile([P, SUP], mm_dt, tag="t2", name="t2")
                    qE = qp[j][:, 0, c10:c11]
                    qO = qp[j][:, 1, c10:c11]
                    nc.vector.scalar_tensor_tensor(t0[:], psA[:], 0.0, ct[:],
                                                   ALU.max, ALU.mult)
                    nc.gpsimd.tensor_tensor(t1[:], xsO[:], st[:], ALU.mult)
                    nc.vector.tensor_tensor(qE, t0[:], t1[:], ALU.subtract)
                    nc.gpsimd.tensor_tensor(t2[:], xsO[:], ct[:], ALU.mult)
                    t3 = ppr.tile([P, SUP], F32, tag="tr", name="t3")
                    nc.vector.scalar_tensor_tensor(t3[:], psA[:], 0.0, st[:],
                                                   ALU.max, ALU.mult)
                    nc.vector.tensor_tensor(qO, t3[:], t2[:], ALU.add)

                for j in range(nJ):
                    if p1_next is not None:
                        p1_step(j)
                    p3_step(2 * j)
                    p3_step(2 * j + 1)
                dec_mms(prevk[0][0], prevk[0][1], stop=True)
                pch = spC.tile([P, nQ * D], mm_dt, tag="pch", name="pch")
                for q in range(nQ):
                    nc.scalar.activation(
                        pch[:, q * D:(q + 1) * D],
                        p_ps[q // 2][:, (q % 2) * D:(q % 2 + 1) * D],
                        AF.Copy, scale=rs4[:, q:q + 1])
                _last_pch[0] = pch

        def emit_ar(l, ts):
            c0, c1 = ts * SUP, (ts + 1) * SUP
            with nc.named_scope(f"l{l}c{ts}_ar"):
                # pch staged by p3 just above (same iteration)
                nc.sync.dma_start(
                    p_loc[l][c0:c1, :].rearrange("(n p) d -> p n d", p=P),
                    _last_pch[0].rearrange("p (n d) -> p n d", n=nQ))
                nc.gpsimd.collective_compute(
                    "AllReduce", ALU.add, replica_groups=RG,
                    ins=[p_loc[l][c0:c1, :]], outs=[p_sum[l][c0:c1, :]])

        def emit_p4(l, ts):
            nxt = (l + 1) % 2
            xbn, xTn = x_bf[nxt], xT_bf[nxt]
            c0, c1 = ts * SUP, (ts + 1) * SUP
            last = (l == NL - 1)
            with nc.named_scope(f"l{l}c{ts}_p4"):
                pin = spC.tile([P, nQ * D], mm_dt, tag="pin", name="pin")
                nc.sync.dma_start(
                    pin[:].rearrange("p (n d) -> p n d", n=nQ),
                    p_sum[l][c0:c1, :].rearrange("(n p) d -> p n d", p=P))
                for q in range(nQ):
                    t = nQ * ts + q
                    xr = spL.tile([P, D], F32, tag="lnr", name="xr", bufs=2)
                    nc.gpsimd.tensor_tensor(xr[:], x_f32[t][:],
                                            pin[:, q * D:(q + 1) * D],
                                            ALU.add)
                    nm_, rs = ln_stats_nm(xr[:])
                    if not last:
                        nc.gpsimd.tensor_scalar(x_f32[t][:], xr[:], nm_[:],
                                                rs[:], ALU.add, ALU.mult)
                    nc.gpsimd.tensor_scalar(xbn[t][:], xr[:], nm_[:],
                                            rs[:], ALU.add, ALU.mult)
                    for d in range(nD):
                        nc.sync.dma_start(
                            xTn[d][:, t * P:(t + 1) * P],
                            xbn[t][:, d * P:(d + 1) * P], transpose=True)

        def emit_final(ts):
            xTf = xT_bf[NL % 2]
            with nc.named_scope(f"fin{ts}"):
                och = spC.tile([P, nQ * V], F32, tag="och", name="och")
                o_ps = [ppw.tile([P, 2 * V], F32, tag="wide", name=f"o{h}")
                        for h in range(nQ // 2)]
                for q in range(nQ):
                    t = nQ * ts + q
                    dst = o_ps[q // 2][:, (q % 2) * V:(q % 2 + 1) * V]
                    for d in range(nD):
                        nc.tensor.matmul(dst,
                                         xTf[d][:, t * P:(t + 1) * P],
                                         wlm_sb[d][:],
                                         start=(d == 0 and q % 2 == 0),
                                         stop=(d == nD - 1 and q % 2 == 1))
                for q in range(nQ):
                    nc.scalar.activation(
                        och[:, q * V:(q + 1) * V],
                        o_ps[q // 2][:, (q % 2) * V:(q % 2 + 1) * V], AF.Copy)
                nc.sync.dma_start(
                    out_d[ts * SUP:(ts + 1) * SUP, :].rearrange(
                        "(n p) v -> p n v", p=P),
                    och[:].rearrange("p (n v) -> p n v", n=nQ))

        # ---- flattened chunk pipeline, p4 delayed by one chunk ----
        n_chunks = NL * nTS

        def emit_delayed(i):
            pl, pts = divmod(i, nTS)
            emit_p4(pl, pts)
            if pl == NL - 1:
                emit_final(pts)

        if nTS == 1:
            for i in range(n_chunks):
                if i >= 1:
                    emit_delayed(i - 1)
                emit_p1(i, 0)
                emit_p2(i, 0)
                emit_p3(i, 0)
                emit_ar(i, 0)
        else:
            # p2 leads each iteration (its qrT was roped last iteration);
            # p1 of the NEXT chunk fills the PE while p2's ykvT stats and
            # p3's operands settle.
            emit_p1(0, 0)
            for i in range(n_chunks):
                l, ts = divmod(i, nTS)
                emit_p2(l, ts)
                nxt = divmod(i + 1, nTS) if i + 1 < n_chunks else None
                emit_p13(l, ts, nxt)
                emit_ar(l, ts)
                if i >= 1:
                    emit_delayed(i - 1)
        emit_delayed(n_chunks - 1)

        for cm in reversed(_cms):
            cm.__exit__(None, None, None)
        for f in reversed(_keep):
            f()
        _keep.clear()

    nc.compile()
    return nc


def host_inputs(idx, embed, encoder, encoder_v, decoder, lm_head, cfg,
                mm_dt=BF16):
    """Build the 8 per-core input maps (host-side prep is O(MB) copies)."""
    T, D, N, NL, V = cfg["T"], cfg["D"], cfg["N"], cfg["NL"], cfg["V"]
    NH, B = cfg["NH"], cfg["B"]
    np_mm = np.dtype(mybir.dt.np(mm_dt))

    idx = np.asarray(idx)
    embed = np.asarray(embed, dtype=np.float32)
    encoder = np.asarray(encoder, dtype=np.float32)
    encoder_v = np.asarray(encoder_v, dtype=np.float32)
    decoder = np.asarray(decoder, dtype=np.float32)
    lm_head = np.asarray(lm_head, dtype=np.float32)

    # initial x = ln(embed[idx]) in f32 (cheap: B*T*D)
    e = embed[idx]  # (B, T, D)
    mu = e.mean(-1, keepdims=True)
    var = ((e - mu) ** 2).mean(-1, keepdims=True)
    x0 = ((e - mu) / np.sqrt(var + EPS)).astype(np.float32)

    # rope tables in pair-permuted transposed layout [N/2, T]
    theta = np.float32(2.0 ** 16)
    q = (np.floor(np.arange(N, dtype=np.float32) / 2.0) * 2.0).astype(np.float32)
    freqs = (1.0 / (theta ** (q / np.float32(N))) /
             np.float32(2.0 * math.pi)).astype(np.float32)
    fp = freqs[0::2]  # (N/2,)
    ph = fp[:, None] * np.arange(T, dtype=np.float32)[None, :]
    pm = ((ph % np.float32(1.0)) * np.float32(2.0 * math.pi)).astype(np.float32)
    ctab = np.cos(pm).astype(np_mm)
    stab = np.sin(pm).astype(np_mm)

    perm = np.concatenate([np.arange(0, N, 2), np.arange(1, N, 2)])
    maskt = np.triu(np.ones((P, P), np.float32), k=1).astype(np_mm)  # s < t
    
    in_maps = []
    for c in range(B * NH):
        b, h = c // NH, c % NH
        x0b = x0[b].astype(np_mm)
        wencv_perm = encoder_v[h][:, perm]  # (D, N)
        wencvp = np.stack([wencv_perm[0:P, :], wencv_perm[P:2 * P, :]],
                          axis=1).reshape(P, 2 * N).astype(np_mm)
        in_maps.append({
            "x0": x0[b],
            "x0b": x0b,
            "x0t": np.ascontiguousarray(x0b.T),
            "wenc": encoder[h][:, perm].astype(np_mm),
            "wencv": wencvp,
            "wdec": decoder[h * N:(h + 1) * N, :][perm, :].astype(np_mm),
            "wlm": lm_head.astype(np_mm),
            "ctab": ctab,
            "stab": stab,
            "maskt": maskt,
        })
    return in_maps


_NC_CACHE = {}


def _get_nc(cfg_key, cfg, mm_dt, n_cores):
    if cfg_key not in _NC_CACHE:
        _NC_CACHE[cfg_key] = build_nc(cfg, mm_dt=mm_dt, n_cores=n_cores)
    return _NC_CACHE[cfg_key]


def kernel(idx, embed, encoder, encoder_v, decoder, lm_head):
    cfg = FULL_CFG
    NH, B = cfg["NH"], cfg["B"]
    n_cores = B * NH
    in_maps = host_inputs(idx, embed, encoder, encoder_v, decoder, lm_head, cfg)
    nc = _get_nc("full_bf16", cfg, BF16, n_cores)
    res = run_bass_kernel_spmd(nc, in_maps, core_ids=list(range(n_cores)))
    out = np.stack([np.asarray(res.results[b * NH]["out"], dtype=np.float32)
                    for b in range(B)], axis=0)
    return out
